# revision 55
# baseline (speedup 1.0000x reference)
"""ClassBalancedSupConLoss on 8 TRN2 NeuronCores (Bass/Tile).

Sharding: the BANK is column-sharded across the 8 cores (2048 cols each,
class-balanced quotas q_c = floor(mcnt_c/8)), every core holds the full
(class-sorted) batch as matmul columns plus its own 256 anchors (merged
into one [anchors | gT | emb] input tensor).

Key idea vs the old kernel: the denominator only needs OTHER-class bank
terms, so per anchor tile we matmul + exp ONLY the complement-class
columns (~2/3 of the slice), packed contiguously in PSUM by slicing the
moving operand.  The ACT accumulator total of that range IS the bank
denominator contribution directly -- no segment reduces at all:

  pure tile (class c): psum <- [other segments packed], one Exp with
    accum_out = den col.  ACT cost ~1.37k elem/lane, zero DVE.
  straddle tile (classes a<b, split row r): psum <- [S_b | S_other |
    S_a]; compl(a) is the prefix, compl(b) the suffix; two
    partition-sliced Exps with separate accum cols.
  bb tiles: full 2048 batch cols, accum total; host does T - selfe
    with selfe computed from the same quantized diag (prelude).

Work split: NOFF of the pure tiles run on the DVE instead (Schraudolph
int16/bf16 bit-trick convert + one bf16 range-sum), placed at odd
emission slots so they rotate through the other PSUM buffer than the
ACT-consumed tiles.  Positives come from the g-trick (e_i . g_c) in the
prelude, as before.

Numerics: matmul inputs fp8 e4m3, exp outputs bf16 (ACT) / int16-bitcast
(DVE), device sums fp32, host assembly fp64.
"""

import os
import numpy as np

import concourse.bass as bass  # noqa: F401
from concourse import bacc
import concourse.mybir as mybir
import concourse.tile as tile
from concourse.bass_utils import run_bass_kernel_spmd

B, D, M, C = 2048, 128, 16384, 3
NCORES = 8
APC = B // NCORES          # own anchors per core = 256
NT = B // 128              # anchor tiles = 16 (all anchors)
NOWN = APC // 128          # own anchor tiles = 2
CH = 512                   # matmul free chunk (one PSUM bank)
W = 2048                   # bank slice cols per core
BASE_TEMP = 0.07

F32 = mybir.dt.float32
BF16 = mybir.dt.bfloat16
I16 = mybir.dt.int16
AF = mybir.ActivationFunctionType
ALU = mybir.AluOpType
AX = mybir.AxisListType

MM_MODE = os.environ.get("SUPCON_MM_MODE", "f8")
WARMUP = int(os.environ.get("SUPCON_WARMUP", "0"))
NOFF = int(os.environ.get("SUPCON_OFFLOAD", "5"))
GPSR = os.environ.get("SUPCON_GPSR", "0") == "1"   # gpsimd dve-tile reduces

# bf16-domain Schraudolph exp: bitcast(int16(A16*y + B16)) ~= e^y
A16 = 128.0 / np.log(2.0)
B16_BASE = 127.0 * 128.0

LAST_EXEC_TIME_NS = None   # set by kernel() when SUPCON_TRACE=1

# oout column layout (per core, [128, OC] fp32).  bb1's two columns are
# last so the main output DMA (cols [0:OC_BB1)) can fire before bb1
# completes; a tiny second DMA carries the rest.
OC_DEN = 0                 # 16: per-tile bank den contribution (or piece-1
                           #     total T1 for straddle tiles)
OC_DEN2 = 16               # 2 straddles x 2: S_b-head and S_a segment sums
OC_DEN3 = 20               # 2: straddle small-piece bitcast sums (P2)
OC_BBT = 22                # bb0 x 2 pieces
OC_SELFE = 24              # 2
OC_RAW3 = 26               # 2 own tiles x 3 = 6
OC_SDIAG = 32              # 2
OC_BB1 = 34                # bb1 x 2 pieces (kept last)
OC = 36

PW_BIG = 1536              # wide PSUM piece (3 banks, 2 buffers)
PW_SM = 512                # small PSUM piece (1 bank, 2 buffers)
MMW = int(os.environ.get("SUPCON_MMW", "512"))   # matmul moving width


def _install_trace_shim():
    """Register the NTFF profile hook that this image's antenv lacks."""
    import sys
    import types
    import ctypes
    import contextlib

    try:
        from antenv.axon_hooks import get_axon_ntff_profile_hook  # noqa: F401
        return True  # real module exists
    except ImportError:
        pass

    so_path = "/opt/axon/libaxon_pjrt.so"
    if not os.path.exists(so_path):
        return False
    lib = ctypes.CDLL(so_path)
    if not hasattr(lib, "axon_start_nrt_profile"):
        return False
    lib.axon_start_nrt_profile.argtypes = [
        ctypes.POINTER(ctypes.c_int64),
        ctypes.c_size_t,
    ]
    lib.axon_start_nrt_profile.restype = ctypes.c_int64
    lib.axon_stop_nrt_profile.argtypes = [ctypes.c_char_p]
    lib.axon_stop_nrt_profile.restype = ctypes.c_int64

    @contextlib.contextmanager
    def _hook(output_dir, device_ids):
        import jax

        jax.devices()
        if device_ids:
            ids = (ctypes.c_int64 * len(device_ids))(*device_ids)
            rc = lib.axon_start_nrt_profile(ids, len(device_ids))
        else:
            rc = lib.axon_start_nrt_profile(None, 0)
        if rc != 0:
            raise RuntimeError(f"axon_start_nrt_profile rc={rc}")
        try:
            yield
        finally:
            n = lib.axon_stop_nrt_profile(str(output_dir).encode())
            print(f"profile: {n} file(s) written to {output_dir}", file=sys.stderr)

    _state = {"hook": _hook}
    mod = types.ModuleType("antenv.axon_hooks")
    mod.get_axon_ntff_profile_hook = lambda: _state["hook"]
    mod.set_axon_ntff_profile_hook = lambda h: _state.update(hook=h)
    sys.modules["antenv.axon_hooks"] = mod
    import antenv

    antenv.axon_hooks = mod

    import concourse.bass_utils as bu

    bu.upload_artifacts = lambda tmpdir: tmpdir
    return True


def _tile_plan(q, tile_cls, strad_r):
    """Per-tile PSUM packing plan against the duplicated-segment bankT
    layout [S0|S1|S2|S0]: every class complement is one contiguous span.

    Returns for each tile t a dict with
      spans: list of (psum_off, bank_off, len) matmul segments, cut at
             PSUM 512 boundaries and span boundaries
      w: total packed width
      segs: for straddle tiles, the two (f0, f1) PSUM ranges whose bf16
            sums (S_b then S_a) the host subtracts from the full total;
            None for pure tiles
    """
    q = [int(x) for x in q]
    off = [0, q[0], q[0] + q[1], q[0] + q[1] + q[2]]  # S0 S1 S2 S0'
    compl = {  # (start, len) of each class's complement, one span
        0: (off[1], q[1] + q[2]),
        1: (off[2], q[2] + q[0]),
        2: (off[0], q[0] + q[1]),
    }
    plans = []
    for t in range(NT):
        c = tile_cls[t]
        if c is not None:
            raw = [compl[c]] if q[(c + 1) % 3] + q[(c + 2) % 3] > 0 else []
            segs = None
        else:
            a, b, _ = strad_r[t]
            # psum = [compl(b) | S_b]; reduce ranges: S_b (suffix) and the
            # position of segment a inside compl(b)
            cb = compl[b]
            raw = [cb, (off[b], q[b])]
            pos_a = (off[a] if b != 1 or a != 0 else off[3]) - cb[0]
            segs = [(cb[1], cb[1] + q[b]), (pos_a, pos_a + q[a])]
        spans = []
        p = 0
        for (boff, blen) in raw:
            s = 0
            while s < blen:
                take = min(blen - s, MMW - (p % MMW))
                spans.append((p, boff + s, take))
                p += take
                s += take
        plans.append({"spans": spans, "w": p, "segs": segs})
    return plans


def _build(plans, off_tiles, emit_order, mm_mode, WB):
    import ml_dtypes  # noqa: F401

    in_dt = mybir.dt.float8e4 if mm_mode == "f8" else BF16

    AW = APC + 8               # anchor block: 256 own + 3 gT + 5 pad
    IW = AW + W                # one merged input: [anchors | batch emb]
    NV = 4 * NT + 4 * NOWN + 128

    nc = bacc.Bacc()
    inp_d = nc.declare_dram_parameter("inp", [D, IW], in_dt, isOutput=False)
    bankT_d = nc.declare_dram_parameter("bankT", [D, WB], in_dt, isOutput=False)
    vecs_d = nc.declare_dram_parameter("vecs", [128, NV], F32, isOutput=False)
    oout_d = nc.declare_dram_parameter("oout", [128, OC], F32, isOutput=True)
    # class-2 complement = bankT [0:c2) -- the openers' columns
    c2 = WB // 2
    for pl in plans:
        if pl["segs"] is None and pl["spans"] and pl["spans"][0][1] == 0:
            c2 = sum(s[2] for s in pl["spans"])
            break

    with tile.TileContext(nc) as tc:
        with (
            tc.tile_pool(name="big", bufs=1) as bigp,
            tc.tile_pool(name="sm", bufs=1) as smp,
            tc.tile_pool(name="scr", bufs=3) as scrp,
            tc.tile_pool(name="i16", bufs=3) as i16p,
            tc.tile_pool(name="ps", bufs=2, space="PSUM") as psp,
            tc.tile_pool(name="pp", bufs=2, space="PSUM") as pps,
        ):
            inp_t = bigp.tile([D, IW], in_dt, tag="inp")
            bank_t = bigp.tile([D, WB], in_dt, tag="bankT")

            def own(t):
                return inp_t[:, t * 128:(t + 1) * 128]

            def ecol(a, b):
                return inp_t[:, AW + a:AW + b]
            vecs_t = smp.tile([128, NV], F32, tag="vecs")
            o = [0]

            def vslice(w):
                a = o[0]; o[0] += w
                return vecs_t[:, a:a + w]
            invt_t = vslice(NT)
            ninvt_t = vslice(NT)
            invo_t = vslice(NOWN)
            ninvo_t = vslice(NOWN)
            sA_t = vslice(NT)
            sB_t = vslice(NT)
            sAo_t = vslice(NOWN)
            sBo_t = vslice(NOWN)
            eye_t = vslice(128)
            junkx_t = bigp.tile([128, CH], in_dt, tag="junkx")

            oout_t = smp.tile([128, OC], F32, tag="oout")
            eyemul = smp.tile([128, 128], F32, tag="eyemul")
            warm = smp.tile([128, 1], F32, tag="warm")
            sdiag = [smp.tile([128, 1], F32, tag=f"sdiag{t}", name=f"sdiag{t}")
                     for t in range(NOWN)]

            # DMA issues first on each queue so transfers begin ASAP; the
            # exp table load + warm activation follow on the scalar queue
            # and run during the transfer window.  inp is split so the
            # anchor block (prelude operands) lands first; bankT is split
            # at c2 so the class-2 openers' columns land first.
            nc.scalar.dma_start(out=vecs_t[:], in_=vecs_d[:])
            nc.sync.dma_start(out=inp_t[:, 0:AW], in_=inp_d[:, 0:AW])
            nc.scalar.dma_start(out=bank_t[:, 0:c2], in_=bankT_d[:, 0:c2])
            nc.sync.dma_start(out=inp_t[:, AW:IW], in_=inp_d[:, AW:IW])
            nc.scalar.dma_start(out=bank_t[:, c2:WB], in_=bankT_d[:, c2:WB])

            nc.vector.memset(oout_t[:], 0.0)
            nc.vector.memset(junkx_t[:, 0:1], 0.0)
            nc.scalar.activation(warm[:], junkx_t[:, 0:1], AF.Exp)

            # PE warmup on garbage operands (HAM clock-gate opener): junk
            # matmuls fill the queue-start -> first-real-matmul window so
            # the 3.4us activity window fires before the stream begins
            if WARMUP:
                junkw_t = bigp.tile([128, 128], in_dt, tag="junkw")
                nc.vector.memset(junkw_t[:], 0.0)
                warm_ps = pps.tile([128, PW_SM], F32, tag="sm",
                                   name="warm_ps")
                for wq in range(WARMUP):
                    nc.tensor.matmul(
                        warm_ps[:, (wq % 4) * 128:(wq % 4) * 128 + 128],
                        junkw_t[:], junkw_t[:], start=True, stop=True,
                    )

            # prelude: self-similarity diag + positives row-sums.  Runs
            # first so the selfe exps land in ACT's otherwise-idle DMA-wait
            # window; its DVE consumers finish before bb1 needs the slot
            post_ps = pps.tile([128, PW_SM], F32, tag="sm", name="post_ps")
            for t in range(NOWN):
                nc.tensor.matmul(
                    post_ps[:, t * 128:(t + 1) * 128], own(t), own(t),
                    start=True, stop=True,
                )
            for t in range(NOWN):
                nc.tensor.matmul(
                    post_ps[:, 256 + t * C:256 + (t + 1) * C], own(t),
                    inp_t[:, APC:APC + C], start=True, stop=True,
                )
            for t in range(NOWN):
                nc.vector.tensor_mul(
                    eyemul[:], post_ps[:, t * 128:(t + 1) * 128], eye_t[:])
                nc.vector.reduce_sum(sdiag[t][:], eyemul[:], axis=AX.X)
                nc.scalar.activation(
                    oout_t[:, OC_SELFE + t:OC_SELFE + t + 1], sdiag[t][:],
                    AF.Exp, bias=ninvo_t[:, t:t + 1], scale=invo_t[:, t:t + 1],
                )
                nc.vector.tensor_copy(
                    out=oout_t[:, OC_SDIAG + t:OC_SDIAG + t + 1], in_=sdiag[t][:])
            nc.vector.tensor_copy(
                out=oout_t[:, OC_RAW3:OC_RAW3 + NOWN * C],
                in_=post_ps[:, 256:256 + NOWN * C])

            def emit_bb(t):
                """bb chunk for own tile t.  The batch block is rotated so
                this core's own (self-term) columns sit in [0:256): the
                1536-wide ACT piece carries the self terms exactly, the
                512-wide tail goes through the small pool as a Schraudolph
                convert + bitcast sum on the DVE."""
                scr = scrp.tile([128, W], BF16, tag="scr", name=f"bbs{t}")
                base = OC_BBT if t == 0 else OC_BB1
                ps = psp.tile([128, PW_BIG], F32, tag="big", name=f"bb{t}")
                a = 0
                while a < PW_BIG:
                    wdt = min(MMW, PW_BIG - a)
                    nc.tensor.matmul(
                        ps[:, a:a + wdt], own(t),
                        inp_t[:, AW + a:AW + a + wdt],
                        start=True, stop=True,
                    )
                    a += wdt
                nc.scalar.activation(
                    scr[:, 0:PW_BIG], ps[:, 0:PW_BIG], AF.Exp,
                    bias=ninvo_t[:, t:t + 1], scale=invo_t[:, t:t + 1],
                    accum_out=oout_t[:, base:base + 1])
                ps2 = pps.tile([128, PW_SM], F32, tag="sm", name=f"bb{t}s")
                nc.tensor.matmul(
                    ps2[:], own(t), inp_t[:, AW + PW_BIG:AW + W],
                    start=True, stop=True,
                )
                i16 = i16p.tile([128, W], I16, tag="i16", name=f"bb{t}i")
                nc.vector.tensor_scalar(
                    out=i16[:, 0:PW_SM], in0=ps2[:],
                    scalar1=sAo_t[:, t:t + 1], scalar2=sBo_t[:, t:t + 1],
                    op0=ALU.mult, op1=ALU.add)
                nc.vector.reduce_sum(
                    oout_t[:, base + 1:base + 2],
                    i16[:, 0:PW_SM].bitcast(BF16), axis=AX.X)

            def mm_piece(ps, t, v0, v1):
                """Matmuls of tile t's packed spans clipped to [v0, v1)."""
                lhs = ecol(t * 128, (t + 1) * 128)
                for (poff, boff, ln) in plans[t]["spans"]:
                    a, b = max(poff, v0), min(poff + ln, v1)
                    if a >= b:
                        continue
                    nc.tensor.matmul(
                        ps[:, a - v0:b - v0], lhs,
                        bank_t[:, boff + a - poff:boff + b - poff],
                        start=True, stop=True,
                    )

            def emit_bank_act(t, si):
                """ACT-path bank chunk: big piece (w<=1536) or big+small
                for straddle tiles, Exp with accum_out = den col(s);
                straddle segment sums on the DVE from the bf16 out."""
                plan = plans[t]
                w = plan["w"]
                scr = scrp.tile([128, W], BF16, tag="scr", name=f"scr{t}")
                ps = psp.tile([128, PW_BIG], F32, tag="big", name=f"ps{t}")
                v1 = min(w, PW_BIG)
                mm_piece(ps, t, 0, v1)
                nc.scalar.activation(
                    scr[:, 0:v1], ps[:, 0:v1], AF.Exp,
                    bias=ninvt_t[:, t:t + 1], scale=invt_t[:, t:t + 1],
                    accum_out=oout_t[:, OC_DEN + t:OC_DEN + t + 1])
                if w > PW_BIG:
                    # straddle tail through the small pool on the DVE
                    ps2 = pps.tile([128, PW_SM], F32, tag="sm",
                                   name=f"ps{t}s")
                    mm_piece(ps2, t, PW_BIG, w)
                    i16 = i16p.tile([128, W], I16, tag="i16", name=f"i{t}s")
                    nc.vector.tensor_scalar(
                        out=i16[:, 0:w - PW_BIG], in0=ps2[:, 0:w - PW_BIG],
                        scalar1=sA_t[:, t:t + 1], scalar2=sB_t[:, t:t + 1],
                        op0=ALU.mult, op1=ALU.add)
                    nc.vector.reduce_sum(
                        oout_t[:, OC_DEN3 + si:OC_DEN3 + si + 1],
                        i16[:, 0:w - PW_BIG].bitcast(BF16), axis=AX.X)
                if plan["segs"] is not None:
                    for j, (f0, f1) in enumerate(plan["segs"]):
                        f1c = min(f1, PW_BIG)
                        if f0 >= f1c:
                            continue
                        nc.vector.reduce_sum(
                            oout_t[:, OC_DEN2 + 2 * si + j:
                                   OC_DEN2 + 2 * si + j + 1],
                            scr[:, f0:f1c], axis=AX.X)

            def emit_bank_dve(t):
                """DVE-path bank chunk: three <=512 sub-chunks through the
                small pool, Schraudolph converts into one i16 tile, then a
                single bf16 range-sum."""
                w = plans[t]["w"]
                i16 = i16p.tile([128, W], I16, tag="i16", name=f"i16{t}")
                for v0 in range(0, w, PW_SM):
                    v1 = min(w, v0 + PW_SM)
                    ps = pps.tile([128, PW_SM], F32, tag="sm",
                                  name=f"ps{t}_{v0}")
                    mm_piece(ps, t, v0, v1)
                    nc.vector.tensor_scalar(
                        out=i16[:, v0:v1], in0=ps[:, 0:v1 - v0],
                        scalar1=sA_t[:, t:t + 1], scalar2=sB_t[:, t:t + 1],
                        op0=ALU.mult, op1=ALU.add)
                red = nc.gpsimd if GPSR else nc.vector
                red.reduce_sum(
                    oout_t[:, OC_DEN + t:OC_DEN + t + 1],
                    i16[:, 0:w].bitcast(BF16), axis=AX.X)

            # bank tiles: ACT tiles rotate the big pool, DVE tiles the
            # small pool; interleaved so both engines stream concurrently.
            # bb0 is emitted after two bank tiles (the bank DMA lands
            # earlier than the batch block, so bank tiles open the stream)
            strad_seen = 0
            for j, t in enumerate(emit_order):
                if j == 2:
                    emit_bb(0)
                if t in off_tiles:
                    emit_bank_dve(t)
                else:
                    si = None
                    if plans[t]["segs"] is not None:
                        si = strad_seen
                        strad_seen += 1
                    emit_bank_act(t, si)

            # bb1 last; the main output DMA (everything except bb1's two
            # columns) fires while bb1 is still in flight, a tiny second
            # DMA carries bb1's columns
            nc.sync.dma_start(out=oout_d[:, 0:OC_BB1], in_=oout_t[:, 0:OC_BB1])
            emit_bb(1)
            nc.sync.dma_start(out=oout_d[:, OC_BB1:OC], in_=oout_t[:, OC_BB1:OC])

    nc.compile()
    return nc


def kernel(embeddings, labels, bank_embs, bank_labels, class_temps):
    global LAST_EXEC_TIME_NS
    import ml_dtypes

    f8 = ml_dtypes.float8_e4m3
    in_np = f8 if MM_MODE == "f8" else ml_dtypes.bfloat16

    emb = np.asarray(embeddings, dtype=np.float32)
    bank = np.asarray(bank_embs, dtype=np.float32)
    lab = np.asarray(labels).astype(np.int64).ravel()
    blab = np.asarray(bank_labels).astype(np.int64).ravel()
    ct = np.asarray(class_temps, dtype=np.float32).ravel()

    # sort batch and bank by class
    bord = np.argsort(lab, kind="stable")
    slab = lab[bord]
    emb_s = emb[bord]                                  # [B, D] f32, sorted
    cnt = np.bincount(lab, minlength=C)
    mord = np.argsort(blab, kind="stable")
    bank_s = bank[mord]
    mcnt = np.bincount(blab, minlength=C)

    # per-core class quotas; <=7*C leftover cols folded in on the host
    q = (mcnt // NCORES).astype(np.int64)              # [3]
    assert int(q.sum()) <= W
    cls_off = np.concatenate([[0], np.cumsum(mcnt)[:-1]])

    # anchor-tile purity (compile-time, same for all cores)
    tile_cls = []
    strad_r = {}
    for t in range(NT):
        seg = slab[t * 128:(t + 1) * 128]
        c_lo, c_hi = int(seg[0]), int(seg[-1])
        if c_lo == c_hi:
            tile_cls.append(c_lo)
        else:
            tile_cls.append(None)
            r = int(np.searchsorted(seg, c_lo, side="right"))
            assert seg[r] == c_hi, "tile straddles >2 classes"
            strad_r[t] = (c_lo, c_hi, r)

    plans = _tile_plan(q, tile_cls, strad_r)

    # quantized operands (shared by device and host-side corrections)
    embq = emb_s.astype(in_np)                         # [B, D]
    bankq = bank_s.astype(in_np)
    embq_f = embq.astype(np.float32)
    bankq_f = bankq.astype(np.float32)
    g = np.stack([emb_s[slab == c].sum(axis=0) for c in range(C)], axis=1)
    gq = g.astype(in_np)                               # [D, 3]

    inv_t_all = (1.0 / ct[slab]).astype(np.float32)    # [B] per sorted anchor

    # DVE-offloaded pure tiles, interleaved at odd emission slots
    pure = [t for t in range(NT) if tile_cls[t] is not None]
    noff = min(NOFF, len(pure), NT // 2)
    off_tiles = (set(pure[int(i)] for i in
                     np.linspace(0, len(pure) - 1, noff).round())
                 if noff > 0 else set())
    # class-2-pure tiles consume the earliest bankT columns, so they can
    # start before the bank DMA finishes; class-1 tiles need the
    # duplicated tail and go last.  Straddle tiles (whose segment sums
    # ride the DVE) are woven between, away from the DVE tiles, which
    # take every third slot so their convert+reduce load spreads out.
    c2s = [t for t in range(NT) if t not in off_tiles and tile_cls[t] == 2]
    c0s = [t for t in range(NT) if t not in off_tiles and tile_cls[t] == 0]
    c1s = [t for t in range(NT) if t not in off_tiles and tile_cls[t] == 1]
    strads = [t for t in range(NT) if tile_cls[t] is None]
    mids = c0s[:1] + strads[:1] + c0s[1:2] + strads[1:] + c0s[2:]
    on_list = c2s + mids + c1s
    off_list = [t for t in range(NT) if t in off_tiles]
    emit_order = []
    ia = ib = 0
    for j in range(NT):
        if j % 3 == 1 and ib < len(off_list):
            emit_order.append(off_list[ib]); ib += 1
        elif ia < len(on_list):
            emit_order.append(on_list[ia]); ia += 1
        else:
            emit_order.append(off_list[ib]); ib += 1

    # per-class Schraudolph bias tuning: pick corr_c that zeroes the mean
    # relative error over the y distribution of this class's logits
    corr_cls = np.zeros(C)
    if True:
        sgrid = np.linspace(-4.0, 4.0, 4001) / np.sqrt(D)
        wpdf = np.exp(-0.5 * (sgrid * np.sqrt(D)) ** 2)
        for c in range(C):
            it = 1.0 / float(ct[c])
            y = it * (sgrid - 1.0)
            exact = np.exp(y)
            best, bestv = 0.0, np.inf
            for corr in np.linspace(0.0, 12.0, 121):
                i16v = np.clip(np.rint(A16 * y + B16_BASE - corr), 0, 32767)
                approx = i16v.astype(np.int16).view(ml_dtypes.bfloat16).astype(np.float64)
                bias = abs(np.sum(wpdf * approx) / np.sum(wpdf * exact) - 1.0)
                if bias < bestv:
                    best, bestv = corr, bias
            corr_cls[c] = best

    WB = int(q.sum() + q[0])                           # [S0|S1|S2|S0']
    nc = _build(plans, off_tiles, emit_order, MM_MODE, WB)

    eye128 = np.eye(128, dtype=np.float32)
    embT = np.ascontiguousarray(embq.T)                # [D, B], shared
    invt_cols = np.ascontiguousarray(inv_t_all.reshape(NT, 128).T)
    sA_cols = (A16 * invt_cols).astype(np.float32)
    corr_all = corr_cls[slab]
    sB_all = (B16_BASE - corr_all - A16 * inv_t_all.astype(np.float64))
    sB_cols = np.ascontiguousarray(sB_all.reshape(NT, 128).T).astype(np.float32)
    AW = APC + 8
    in_maps = []
    for k in range(NCORES):
        asl = slice(k * APC, (k + 1) * APC)
        inp = np.zeros((D, AW + W), dtype=in_np)
        inp[:, 0:APC] = embq[asl].T
        inp[:, APC:APC + C] = gq
        rot = (np.arange(B) + k * APC) % B             # own cols first
        inp[:, AW:AW + W] = embT[:, rot]
        bankT = np.zeros((D, WB), dtype=in_np)
        pos = 0
        for c in (0, 1, 2, 0):                         # dup S0 at the end
            sel = bankq[cls_off[c] + k * q[c]: cls_off[c] + (k + 1) * q[c]]
            bankT[:, pos:pos + q[c]] = sel.T
            pos += int(q[c])
        ivo = inv_t_all[asl]
        sBo = (B16_BASE - corr_all[asl]
               - A16 * ivo.astype(np.float64)).astype(np.float32)
        vparts = [
            invt_cols, -invt_cols,
            np.ascontiguousarray(ivo.reshape(NOWN, 128).T),
            np.ascontiguousarray((-ivo).reshape(NOWN, 128).T),
            sA_cols, sB_cols,
            np.ascontiguousarray((A16 * ivo).reshape(NOWN, 128).T),
            np.ascontiguousarray(sBo.reshape(NOWN, 128).T),
        ]
        vecs = np.concatenate(vparts + [eye128], axis=1).astype(np.float32)
        in_maps.append({
            "inp": np.ascontiguousarray(inp),
            "bankT": np.ascontiguousarray(bankT),
            "vecs": np.ascontiguousarray(vecs),
        })

    trace = os.environ.get("SUPCON_TRACE", "0") == "1"
    if trace:
        trace = _install_trace_shim()
    res = run_bass_kernel_spmd(nc, in_maps, core_ids=list(range(NCORES)), trace=trace)
    LAST_EXEC_TIME_NS = res.exec_time_ns

    # ---- host assembly (fp64) ----
    inv64 = inv_t_all.astype(np.float64)
    den = np.zeros(B, dtype=np.float64)
    raw3_own = np.zeros(B, dtype=np.float64)
    sdiag_own = np.zeros(B, dtype=np.float64)
    tidx = np.arange(128)
    strad_order = sorted(strad_r.keys(), key=lambda t: emit_order.index(t))
    for k in range(NCORES):
        oo = np.asarray(res.results[k]["oout"], dtype=np.float64)  # [128, OC]
        for t in range(NT):
            a_idx = t * 128 + tidx
            if tile_cls[t] is not None or t in off_tiles:
                den[a_idx] += oo[:, OC_DEN + t]
            else:
                # rows [0:r) are class a: den = T - S_a; rows [r:) class b:
                # den = T - S_b, with T = T1 + P2 and S_b = S_b_head + P2
                r = strad_r[t][2]
                si = strad_order.index(t)
                T = oo[:, OC_DEN + t] + oo[:, OC_DEN3 + si]
                s_b = oo[:, OC_DEN2 + 2 * si] + oo[:, OC_DEN3 + si]
                s_a = oo[:, OC_DEN2 + 2 * si + 1]
                den[a_idx[:r]] += T[:r] - s_a[:r]
                den[a_idx[r:]] += T[r:] - s_b[r:]
        for t in range(NOWN):
            a_idx = k * APC + t * 128 + tidx            # own anchors
            base = OC_BBT if t == 0 else OC_BB1
            bbT = oo[:, base] + oo[:, base + 1]
            den[a_idx] += bbT - oo[:, OC_SELFE + t]
            sdiag_own[a_idx] = oo[:, OC_SDIAG + t]
            cls = slab[a_idx]
            raw3_own[a_idx] = oo[tidx, OC_RAW3 + t * 3 + cls]

    # leftover (overflow) bank columns, folded in exactly on the host
    ov_cols, ov_cls = [], []
    for c in range(C):
        lo, hi = cls_off[c] + NCORES * q[c], cls_off[c] + mcnt[c]
        for j in range(lo, hi):
            ov_cols.append(j)
            ov_cls.append(c)
    if ov_cols:
        bq = bankq_f[ov_cols]                           # [n_ov, D]
        s_ov = embq_f @ bq.T                            # [B, n_ov]
        terms = np.exp(inv64[:, None] * (s_ov.astype(np.float64) - 1.0))
        mask = slab[:, None] != np.asarray(ov_cls)[None, :]
        den += (terms * mask).sum(axis=1)

    pos_cnt = (cnt[slab] - 1).astype(np.float64)
    pos_sum = raw3_own - sdiag_own
    pos_mean = pos_sum / np.maximum(pos_cnt, 1.0)
    log_denom = inv64 + np.log(den)
    coef = BASE_TEMP * inv64
    loss_i = coef * (log_denom - pos_mean)
    valid = pos_cnt > 0
    n_valid = int(valid.sum())
    loss = (loss_i * valid).sum() / max(n_valid, 1)
    return np.float32(loss)
